# revision 2
# baseline (speedup 1.0000x reference)
"""AutoCorrelation kernel for Trainium2 (8 NeuronCores, SPMD data-parallel over batch).

Math (derived from the reference nn.Module):
  - R = irfft(rfft(Q) * conj(rfft(K))) is a circular cross-correlation; the
    reference reduces it with mean over (heads, ALL lags).  Sum over all lags
    of a circular cross-correlation factorizes:  sum_tau R[tau] =
    (sum_t Q[t]) * (sum_s K[s]).  So the FFT is algebraically unnecessary --
    only column sums of Q and K are needed, and those are linear in the
    column sums of q and k (sum_t(q @ Wq + bq) = (sum_t q) @ Wq + L*bq).
  - The top-k "delays" are channel indices in [0, 64).  The delay aggregation
    sum_i w_i * roll(V, -d_i) commutes with the output projection AND with the
    value projection, so:  out[t] = sum_d coef_d * U[(t+d) % L]  where
    U = v @ (Wv @ Wo), plus bias (bv @ Wo + bo).  The tap sum is a 64-band
    Toeplitz matmul on the tensor engine.
  - The band coefficients sum to exactly 1 (softmax), so the bias can be
    folded into U (out = bands^T (U + bias) = bands^T U + bias) -- the bias
    add rides the PSUM->SBUF cast instead of the conv output path.

Device work:
  phase 1: column sums of q[b], k[b] per core via ones-vector matmuls
           (memory bound; bf16 inputs, fp32 PSUM accumulation)
  phase 2: U = v @ W2 + bias per 128-row tile, then out_i = band1^T U_i +
           band2^T U_{i+1} (circular), stored as bf16 (host upcasts).
           PE warm-up matmuls at kernel start remove the HAM cold-clock
           penalty while the first vT slices are still in flight.
Host work: [8,512]@[512,512] glue matmuls, top-41 of 64, softmax, band build.
"""

import sys

sys.path.insert(0, "/opt/trn_rl_repo")

import numpy as np

import concourse.bass as bass
import concourse.bacc as bacc
import concourse.mybir as mybir
import concourse.tile as tile
from concourse.bass_utils import run_bass_kernel_spmd

B, L, D, H = 8, 4096, 512, 8
DK = D // H          # 64
K_TOP = 41           # min(int(5*log(4096)), 64)
NCORES = 8
F32 = mybir.dt.float32
BF16 = mybir.dt.bfloat16
NP_BF16 = mybir.dt.np(BF16)

# set by test.py to collect HW profiles
PROFILE = False
TRACE_DIR = None
LAST_HW_TIME_NS = {"phase1": None, "phase2": None}

_NC_CACHE = {}


def _make_nc():
    return bacc.Bacc(
        "TRN2", target_bir_lowering=False, debug=False, num_devices=NCORES
    )


def _build_phase1():
    """Per-core: sums[0, :512] = sum_t q[t, :], sums[0, 512:] = sum_t k[t, :].

    q/k arrive as bf16; sums accumulate in fp32 PSUM via ones-vector matmuls.
    DMA plan per stream: 3 x 1 MB + 4 x 0.25 MB tiles -- big early tiles for
    bandwidth, small late tiles so the post-last-byte matmul tail is short.
    """
    nc = _make_nc()
    q = nc.dram_tensor("q", [L, D], BF16, kind="ExternalInput")
    k = nc.dram_tensor("k", [L, D], BF16, kind="ExternalInput")
    sums = nc.dram_tensor("sums", [1, 2 * D], F32, kind="ExternalOutput")

    NBIGSUB = 8              # 1 MB tiles
    NSMSUB = 2               # 0.25 MB tiles
    NBIG = 3                 # rows 0..3072
    NSM = 4                  # rows 3072..4096

    with tile.TileContext(nc) as tc:
        with (
            tc.tile_pool(name="singles", bufs=1) as singles,
            tc.tile_pool(name="qk", bufs=3) as qk_pool,
            tc.tile_pool(name="qks", bufs=4) as qks_pool,
            tc.tile_pool(name="ps", bufs=2, space=bass.MemorySpace.PSUM) as ps_pool,
        ):
            ones = singles.tile([128, 1], BF16)
            nc.any.memset(ones[:], 1.0)

            q_big = q.ap().rearrange("(g p n) d -> g p n d", p=128, n=NBIGSUB)
            k_big = k.ap().rearrange("(g p n) d -> g p n d", p=128, n=NBIGSUB)
            q_sm = q.ap().rearrange("(g p n) d -> g p n d", p=128, n=NSMSUB)
            k_sm = k.ap().rearrange("(g p n) d -> g p n d", p=128, n=NSMSUB)
            SM0 = NBIG * 4           # first small-group index (rows 3072+)

            chunks = []              # (q_tile, k_tile, nsub)
            for g in range(NBIG):
                tq = qk_pool.tile([128, NBIGSUB, D], BF16, tag="ldq", name=f"tq{g}")
                nc.sync.dma_start(tq[:], q_big[g])
                tk = qk_pool.tile([128, NBIGSUB, D], BF16, tag="ldk", name=f"tk{g}")
                nc.scalar.dma_start(tk[:], k_big[g])
                chunks.append((tq, tk, NBIGSUB))
            for s in range(NSM):
                tq = qks_pool.tile([128, NSMSUB, D], BF16, tag="sdq", name=f"sq{s}")
                nc.sync.dma_start(tq[:], q_sm[SM0 + s])
                tk = qks_pool.tile([128, NSMSUB, D], BF16, tag="sdk", name=f"sk{s}")
                nc.scalar.dma_start(tk[:], k_sm[SM0 + s])
                chunks.append((tq, tk, NSMSUB))

            psq = ps_pool.tile([1, D], F32)
            psk = ps_pool.tile([1, D], F32)
            ssb = singles.tile([1, 2 * D], F32)
            nchunks = len(chunks)
            for ci, (tq, tk, nsub) in enumerate(chunks):
                first = ci == 0
                last = ci == nchunks - 1
                for c in range(nsub):
                    nc.tensor.matmul(
                        psq[:1, :],
                        ones[:],
                        tq[:, c, :],
                        start=(first and c == 0),
                        stop=(last and c == nsub - 1),
                    )
                for c in range(nsub):
                    nc.tensor.matmul(
                        psk[:1, :],
                        ones[:],
                        tk[:, c, :],
                        start=(first and c == 0),
                        stop=(last and c == nsub - 1),
                    )
                if last:
                    # q's accumulation finished just above; drain it while
                    # the final k matmuls run
                    nc.vector.tensor_copy(ssb[:1, 0:D], psq[:1, :])
            nc.vector.tensor_copy(ssb[:1, D : 2 * D], psk[:1, :])
            nc.sync.dma_start(sums.ap(), ssb[:])

    nc.compile()
    return nc


def _build_phase2():
    """Per-core: out[128i + t, n] = sum_s band1[s, t] * U_i[s, n]
                                  + sum_s band2[s, t] * U_{i+1 mod 32}[s, n]
    with U_i = v[128i : 128(i+1), :] @ W2 + bias, from host-transposed vT.
    Output stored bf16 (host upcasts); bias folded into U (bands sum to 1).
    """
    nc = _make_nc()
    vT = nc.dram_tensor("vT", [D, L], BF16, kind="ExternalInput")
    bandsd = nc.dram_tensor("bands", [2, 128, 128], BF16, kind="ExternalInput")
    # host-swizzled: w2[p, cg*D + n] = (Wv@Wo)[cg*128 + p, n] (contiguous rows)
    w2d = nc.dram_tensor("w2", [128, 4 * D], BF16, kind="ExternalInput")
    biasd = nc.dram_tensor("bias", [128, D], F32, kind="ExternalInput")
    out = nc.dram_tensor("out", [L, D], BF16, kind="ExternalOutput")
    warm = nc.dram_tensor("warm", [1, 4], F32, kind="ExternalOutput")

    NBLK = L // 128          # 32 tiles / output blocks
    OSUB = 2                 # output blocks per store DMA
    HEADC = 512              # head slice cols per channel group (4 U tiles)
    NHEADT = HEADC // 128
    RESTC = (L - HEADC) // 2  # two rest chunks per channel group
    RESTT = RESTC // 128
    PRE_U = 8                # U tiles emitted before the first conv block
    NWARM = 10               # PE warm-up matmuls (HAM ramp) before real work

    with tile.TileContext(nc) as tc:
        with (
            tc.tile_pool(name="singles", bufs=1) as singles,
            tc.tile_pool(name="usb", bufs=PRE_U + 3) as u_pool,
            tc.tile_pool(name="op", bufs=3) as opool,
            tc.tile_pool(name="wps", bufs=1, space=bass.MemorySpace.PSUM) as wps_pool,
            tc.tile_pool(name="ups", bufs=4, space=bass.MemorySpace.PSUM) as ups_pool,
            tc.tile_pool(name="ops", bufs=2, space=bass.MemorySpace.PSUM) as ops_pool,
        ):
            # ---- PE warm-up: no data deps, fills the HAM ramp while the
            # first vT slices are in flight.  A tiny store keeps it live. ----
            wtile = singles.tile([128, 512], BF16)
            nc.any.memset(wtile[:], 0.0)
            wps = wps_pool.tile([128, 512], F32)
            for j in range(NWARM):
                nc.tensor.matmul(
                    wps[:], wtile[:, 0:128], wtile[:], start=(j == 0), stop=(j == NWARM - 1)
                )
            wsb = singles.tile([1, 4], F32)
            nc.vector.tensor_copy(wsb[:], wps[0:1, 0:4])
            nc.sync.dma_start(warm.ap(), wsb[:])

            # ---- DMA schedule: w2 + 4 small head slices first so the first
            # real matmul starts ~2us after DMA begins; rest of vT lands in
            # two staggered chunks per channel group. ----
            vt_re = vT.ap().rearrange("(c p) t -> c p t", p=128)
            w2_sb = singles.tile([128, 4, D], BF16)
            nc.sync.dma_start(w2_sb[:], w2d.ap().rearrange("p (c n) -> p c n", c=4))
            vth = [singles.tile([128, HEADC], BF16, name=f"vth{cg}") for cg in range(4)]
            nc.scalar.dma_start(vth[0][:], vt_re[0][:, 0:HEADC])
            nc.scalar.dma_start(vth[2][:], vt_re[2][:, 0:HEADC])
            nc.sync.dma_start(vth[1][:], vt_re[1][:, 0:HEADC])
            nc.sync.dma_start(vth[3][:], vt_re[3][:, 0:HEADC])
            band_sb = singles.tile([128, 2, 128], BF16)
            nc.scalar.dma_start(band_sb[:], bandsd.ap().rearrange("b p t -> p b t"))
            bias_sb = singles.tile([128, D], F32)
            nc.scalar.dma_start(bias_sb[:], biasd.ap())
            vtr = [
                [
                    singles.tile([128, RESTC], BF16, name=f"vtr{cg}_{h}")
                    for h in range(2)
                ]
                for cg in range(4)
            ]
            for h in range(2):
                for cg in range(4):
                    ring = nc.sync if cg % 2 == 0 else nc.scalar
                    ring.dma_start(
                        vtr[cg][h][:],
                        vt_re[cg][:, HEADC + h * RESTC : HEADC + (h + 1) * RESTC],
                    )

            out_re = out.ap().rearrange("(g n p) d -> g p n d", p=128, n=OSUB)

            def u_src(i, cg):
                if i < NHEADT:
                    return vth[cg][:, i * 128 : (i + 1) * 128]
                j = i - NHEADT
                h, r = divmod(j, RESTT)
                return vtr[cg][h][:, r * 128 : (r + 1) * 128]

            def u_tile(i):
                ups = ups_pool.tile([128, D], F32, tag="ups", name=f"ups{i}")
                for cg in range(4):
                    nc.tensor.matmul(
                        ups[:],
                        u_src(i, cg),
                        w2_sb[:, cg, :],
                        start=(cg == 0),
                        stop=(cg == 3),
                    )
                # PSUM -> SBUF with the bias folded in and a bf16 downcast
                usb = u_pool.tile([128, D], BF16, tag="usb", name=f"usb{i}")
                nc.vector.tensor_add(usb[:], ups[:], bias_sb[:])
                return usb

            U = {}
            for i in range(PRE_U):
                U[i] = u_tile(i)
            u_first = singles.tile([128, D], BF16)
            nc.vector.tensor_copy(u_first[:], U[0][:])

            ot_tiles = {}
            for i in range(NBLK):
                g, n4 = divmod(i, OSUB)
                if g not in ot_tiles:
                    ot_tiles[g] = opool.tile(
                        [128, OSUB, D], BF16, tag="out", name=f"ot{g}"
                    )
                if i + PRE_U < NBLK:
                    U[i + PRE_U] = u_tile(i + PRE_U)
                u_n = U[i + 1] if i < NBLK - 1 else u_first
                ops = ops_pool.tile([128, D], F32, tag="ops", name=f"ops{i}")
                nc.tensor.matmul(
                    ops[:], band_sb[:, 0, :], U[i][:], start=True, stop=False
                )
                nc.tensor.matmul(
                    ops[:], band_sb[:, 1, :], u_n[:], start=False, stop=True
                )
                del U[i]
                ot = ot_tiles[g]
                nc.scalar.copy(ot[:, n4, :], ops[:])  # ACT: fp32 PSUM -> bf16 SBUF
                if n4 == OSUB - 1:
                    nc.sync.dma_start(out_re[g], ot[:])
                    del ot_tiles[g]

    nc.compile()
    return nc


_RUN_COUNTER = [0]


def _run(nc, in_maps, phase):
    kwargs = {}
    if PROFILE:
        kwargs["trace"] = True
        if TRACE_DIR is not None:
            import os

            _RUN_COUNTER[0] += 1
            d = os.path.join(TRACE_DIR, f"{phase}_{_RUN_COUNTER[0]}")
            os.makedirs(d, exist_ok=True)
            kwargs["tmpdir"] = d
    res = run_bass_kernel_spmd(nc, in_maps, core_ids=list(range(NCORES)), **kwargs)
    LAST_HW_TIME_NS[phase] = res.exec_time_ns
    return res.results


def kernel(q, k, v, Wq, bq, Wk, bk, Wv, bv, Wo, bo):
    q = np.asarray(q, dtype=np.float32)
    k = np.asarray(k, dtype=np.float32)
    v = np.asarray(v, dtype=np.float32)
    Wq, bq, Wk, bk, Wv, bv, Wo, bo = (
        np.asarray(x, dtype=np.float64) for x in (Wq, bq, Wk, bk, Wv, bv, Wo, bo)
    )

    # ---- phase 1: per-batch column sums of q and k (device) ----
    if "p1" not in _NC_CACHE:
        _NC_CACHE["p1"] = _build_phase1()
    q_bf = q.astype(NP_BF16)
    k_bf = k.astype(NP_BF16)
    in_maps = [{"q": q_bf[b], "k": k_bf[b]} for b in range(B)]
    res1 = _run(_NC_CACHE["p1"], in_maps, "phase1")
    sq = np.stack([res1[b]["sums"][0, :D] for b in range(B)]).astype(np.float64)
    sk = np.stack([res1[b]["sums"][0, D:] for b in range(B)]).astype(np.float64)

    # ---- host glue: top-k channel selection + softmax weights ----
    SQ = sq @ Wq + L * bq                       # [B, D]
    SK = sk @ Wk + L * bk
    m = (SQ.reshape(B, H, DK) * SK.reshape(B, H, DK)).sum(axis=1) / (H * L)  # [B, DK]
    mbar = m.mean(axis=0)
    idx = np.argsort(-mbar, kind="stable")[:K_TOP]
    msel = m[:, idx]
    e = np.exp(msel - msel.max(axis=1, keepdims=True))
    w = e / e.sum(axis=1, keepdims=True)        # [B, K_TOP]
    coef = np.zeros((B, DK))
    coef[:, idx] = w

    # Toeplitz bands: out[t] = sum_d coef[d] * U[(t + d) % L]
    s = np.arange(128)[:, None]
    t = np.arange(128)[None, :]
    d1 = s - t
    d2 = s + 128 - t
    m1 = (d1 >= 0) & (d1 < DK)
    m2 = (d2 >= 0) & (d2 < DK)
    bands = np.zeros((B, 2, 128, 128), dtype=np.float64)
    for b in range(B):
        bands[b, 0] = np.where(m1, coef[b][np.clip(d1, 0, DK - 1)], 0.0)
        bands[b, 1] = np.where(m2, coef[b][np.clip(d2, 0, DK - 1)], 0.0)

    W2 = (Wv @ Wo).astype(np.float32)
    bias2 = (bv @ Wo + bo).astype(np.float32)
    bias_rep = np.ascontiguousarray(np.broadcast_to(bias2, (128, D)))
    # swizzle so W2 rows for channel chunk cg sit contiguously per partition
    w2_bf = np.ascontiguousarray(
        W2.reshape(4, 128, D).transpose(1, 0, 2).reshape(128, 4 * D)
    ).astype(NP_BF16)
    bands_bf = bands.astype(NP_BF16)
    vT_bf = np.ascontiguousarray(v.transpose(0, 2, 1)).astype(NP_BF16)  # [B, D, L]

    # ---- phase 2: folded projection + tap aggregation (device) ----
    if "p2" not in _NC_CACHE:
        _NC_CACHE["p2"] = _build_phase2()
    in_maps = [
        {
            "vT": vT_bf[b],
            "bands": np.ascontiguousarray(bands_bf[b]),
            "w2": w2_bf,
            "bias": bias_rep,
        }
        for b in range(B)
    ]
    res2 = _run(_NC_CACHE["p2"], in_maps, "phase2")
    return np.stack([res2[b]["out"] for b in range(B)]).astype(np.float32)


# revision 5
# speedup vs baseline: 1.0087x; 1.0087x over previous
"""AutoCorrelation kernel for Trainium2 (8 NeuronCores, SPMD data-parallel over batch).

Math (derived from the reference nn.Module):
  - R = irfft(rfft(Q) * conj(rfft(K))) is a circular cross-correlation; the
    reference reduces it with mean over (heads, ALL lags).  Sum over all lags
    of a circular cross-correlation factorizes:  sum_tau R[tau] =
    (sum_t Q[t]) * (sum_s K[s]).  So the FFT is algebraically unnecessary --
    only column sums of Q and K are needed, and those are linear in the
    column sums of q and k (sum_t(q @ Wq + bq) = (sum_t q) @ Wq + L*bq).
  - The top-k "delays" are channel indices in [0, 64).  The delay aggregation
    sum_i w_i * roll(V, -d_i) commutes with the output projection AND with the
    value projection, so:  out[t] = sum_d coef_d * U[(t+d) % L]  where
    U = v @ (Wv @ Wo), plus bias (bv @ Wo + bo).  The tap sum is a 64-band
    Toeplitz matmul on the tensor engine.
  - The band coefficients sum to exactly 1 (softmax), so the bias can be
    folded into U (out = bands^T (U + bias) = bands^T U + bias) -- the bias
    add rides the PSUM->SBUF cast instead of the conv output path.

Device work:
  phase 1: column sums of q[b], k[b] per core via ones-vector matmuls
           (memory bound; bf16 inputs, fp32 PSUM accumulation)
  phase 2: U = v @ W2 + bias per 128-row tile, then out_i = band1^T U_i +
           band2^T U_{i+1} (circular), stored as bf16 (host upcasts).
           PE warm-up matmuls at kernel start remove the HAM cold-clock
           penalty while the first vT slices are still in flight.
Host work: [8,512]@[512,512] glue matmuls, top-41 of 64, softmax, band build.
"""

import sys

sys.path.insert(0, "/opt/trn_rl_repo")

import numpy as np

import concourse.bass as bass
import concourse.bacc as bacc
import concourse.mybir as mybir
import concourse.tile as tile
from concourse.bass_utils import run_bass_kernel_spmd

B, L, D, H = 8, 4096, 512, 8
DK = D // H          # 64
K_TOP = 41           # min(int(5*log(4096)), 64)
NCORES = 8
F32 = mybir.dt.float32
BF16 = mybir.dt.bfloat16
NP_BF16 = mybir.dt.np(BF16)

# set by test.py to collect HW profiles
PROFILE = False
TRACE_DIR = None
LAST_HW_TIME_NS = {"phase1": None, "phase2": None}

_NC_CACHE = {}


def _make_nc():
    return bacc.Bacc(
        "TRN2", target_bir_lowering=False, debug=False, num_devices=NCORES
    )


def _build_phase1():
    """Per-core: sums[0, :512] = sum_t q[t, :], sums[0, 512:] = sum_t k[t, :].

    q/k arrive as bf16; sums accumulate in fp32 PSUM via ones-vector matmuls.
    DMA plan per stream: 3 x 1 MB + 4 x 0.25 MB tiles -- big early tiles for
    bandwidth, small late tiles so the post-last-byte matmul tail is short.
    """
    nc = _make_nc()
    q = nc.dram_tensor("q", [L, D], BF16, kind="ExternalInput")
    k = nc.dram_tensor("k", [L, D], BF16, kind="ExternalInput")
    sums = nc.dram_tensor("sums", [1, 2 * D], F32, kind="ExternalOutput")

    NBIGSUB = 4              # 0.5 MB tiles
    NSMSUB = 2               # 0.25 MB tail tiles
    NBIG = 7                 # rows 0..3584
    NSM = 2                  # rows 3584..4096

    with tile.TileContext(nc) as tc:
        with (
            tc.tile_pool(name="singles", bufs=1) as singles,
            tc.tile_pool(name="qk", bufs=NBIG) as qk_pool,
            tc.tile_pool(name="qks", bufs=NSM) as qks_pool,
            tc.tile_pool(name="ps", bufs=2, space=bass.MemorySpace.PSUM) as ps_pool,
        ):
            ones = singles.tile([128, 1], BF16)
            nc.any.memset(ones[:], 1.0)

            q_big = q.ap().rearrange("(g p n) d -> g p n d", p=128, n=NBIGSUB)
            k_big = k.ap().rearrange("(g p n) d -> g p n d", p=128, n=NBIGSUB)
            q_sm = q.ap().rearrange("(g p n) d -> g p n d", p=128, n=NSMSUB)
            k_sm = k.ap().rearrange("(g p n) d -> g p n d", p=128, n=NSMSUB)
            SM0 = NBIG * 2           # first small-group index (rows 3584+)

            chunks = []              # (q_tile, k_tile, nsub)
            for g in range(NBIG):
                tq = qk_pool.tile([128, NBIGSUB, D], BF16, tag="ldq", name=f"tq{g}")
                nc.sync.dma_start(tq[:], q_big[g])
                tk = qk_pool.tile([128, NBIGSUB, D], BF16, tag="ldk", name=f"tk{g}")
                nc.scalar.dma_start(tk[:], k_big[g])
                chunks.append((tq, tk, NBIGSUB))
            for s in range(NSM):
                tq = qks_pool.tile([128, NSMSUB, D], BF16, tag="sdq", name=f"sq{s}")
                nc.sync.dma_start(tq[:], q_sm[SM0 + s])
                tk = qks_pool.tile([128, NSMSUB, D], BF16, tag="sdk", name=f"sk{s}")
                nc.scalar.dma_start(tk[:], k_sm[SM0 + s])
                chunks.append((tq, tk, NSMSUB))

            psq = ps_pool.tile([1, D], F32)
            psk = ps_pool.tile([1, D], F32)
            ssb = singles.tile([1, 2 * D], F32)
            nchunks = len(chunks)
            for ci, (tq, tk, nsub) in enumerate(chunks):
                first = ci == 0
                last = ci == nchunks - 1
                for c in range(nsub):
                    nc.tensor.matmul(
                        psq[:1, :],
                        ones[:],
                        tq[:, c, :],
                        start=(first and c == 0),
                        stop=(last and c == nsub - 1),
                    )
                for c in range(nsub):
                    nc.tensor.matmul(
                        psk[:1, :],
                        ones[:],
                        tk[:, c, :],
                        start=(first and c == 0),
                        stop=(last and c == nsub - 1),
                    )
                if last:
                    # q's accumulation finished just above; drain it while
                    # the final k matmuls run
                    nc.vector.tensor_copy(ssb[:1, 0:D], psq[:1, :])
            nc.vector.tensor_copy(ssb[:1, D : 2 * D], psk[:1, :])
            nc.sync.dma_start(sums.ap(), ssb[:])

    nc.compile()
    return nc


def _build_phase2():
    """Per-core: out[128i + t, n] = sum_s band1[s, t] * U_i[s, n]
                                  + sum_s band2[s, t] * U_{i+1 mod 32}[s, n]
    with U_i = v[128i : 128(i+1), :] @ W2 + bias, from host-transposed vT.
    Output stored bf16 (host upcasts); bias folded into U (bands sum to 1).
    """
    nc = _make_nc()
    vT = nc.dram_tensor("vT", [D, L], BF16, kind="ExternalInput")
    bandsd = nc.dram_tensor("bands", [2, 128, 128], BF16, kind="ExternalInput")
    # host-swizzled halves: w2a[p, n] = W2[p, n]; w2b[p, c*D + n] = W2[(c+1)*128 + p, n]
    w2ad = nc.dram_tensor("w2a", [128, D], BF16, kind="ExternalInput")
    w2bd = nc.dram_tensor("w2b", [128, 3 * D], BF16, kind="ExternalInput")
    biasd = nc.dram_tensor("bias", [128, D], F32, kind="ExternalInput")
    out = nc.dram_tensor("out", [L, D], BF16, kind="ExternalOutput")
    warm = nc.dram_tensor("warm", [1, 4], F32, kind="ExternalOutput")

    NBLK = L // 128          # 32 tiles / output blocks
    OSUB = 2                 # output blocks per store DMA
    # vT arrives per channel group in geometric column levels: the first
    # levels unblock the matmul stream early, later levels amortize DMA
    # overhead while the PE stream provides plenty of runway.
    LEVELS = [512, 512, 1024, 2048]
    PRE_U = 8                # U tiles emitted before the first conv block
    NWARM = 14               # PE warm-up matmuls (HAM ramp) before real work

    with tile.TileContext(nc) as tc:
        with (
            tc.tile_pool(name="singles", bufs=1) as singles,
            tc.tile_pool(name="usb", bufs=PRE_U + 3) as u_pool,
            tc.tile_pool(name="op", bufs=3) as opool,
            tc.tile_pool(name="wps", bufs=1, space=bass.MemorySpace.PSUM) as wps_pool,
            tc.tile_pool(name="ups", bufs=4, space=bass.MemorySpace.PSUM) as ups_pool,
            tc.tile_pool(name="ops", bufs=2, space=bass.MemorySpace.PSUM) as ops_pool,
        ):
            # ---- PE warm-up: no data deps, fills the HAM ramp while the
            # first vT slices are in flight.  A tiny store keeps it live.
            # Small N so the queue drains just as the first real operands
            # land (~1.8us of PE-busy). ----
            wtile = singles.tile([128, 512], BF16)
            nc.any.memset(wtile[:], 0.0)
            wps = wps_pool.tile([128, 512], F32)
            for j in range(NWARM):
                nc.tensor.matmul(
                    wps[:, 0:128],
                    wtile[:, 0:128],
                    wtile[:, 0:128],
                    start=(j == 0),
                    stop=(j == NWARM - 1),
                )
            wsb = singles.tile([1, 4], F32)
            nc.vector.tensor_copy(wsb[:], wps[0:1, 0:4])
            nc.sync.dma_start(warm.ap(), wsb[:])

            # ---- DMA schedule.  Ring FIFOs matter: concurrent transfers on
            # a ring fair-share bandwidth, so the operands of the first
            # matmuls are first and small on BOTH rings. ----
            vt_re = vT.ap().rearrange("(c p) t -> c p t", p=128)
            w2a_sb = singles.tile([128, D], BF16)
            nc.sync.dma_start(w2a_sb[:], w2ad.ap())
            w2b_sb = singles.tile([128, 3, D], BF16)
            nc.scalar.dma_start(
                w2b_sb[:], w2bd.ap().rearrange("p (c n) -> p c n", c=3)
            )
            L0 = LEVELS[0]
            vth = [singles.tile([128, L0], BF16, name=f"vth{cg}") for cg in range(4)]
            nc.sync.dma_start(vth[0][:], vt_re[0][:, 0:L0])
            nc.scalar.dma_start(vth[1][:], vt_re[1][:, 0:L0])
            nc.sync.dma_start(vth[2][:], vt_re[2][:, 0:L0])
            nc.scalar.dma_start(vth[3][:], vt_re[3][:, 0:L0])
            bias_sb = singles.tile([128, D], F32)
            nc.scalar.dma_start(bias_sb[:], biasd.ap())
            band_sb = singles.tile([128, 2, 128], BF16)
            nc.scalar.dma_start(band_sb[:], bandsd.ap().rearrange("b p t -> p b t"))
            vlv = {}                 # (level, cg) -> (tile, col0)
            col = L0
            for lv, width in enumerate(LEVELS[1:], start=1):
                for cg in range(4):
                    t = singles.tile([128, width], BF16, name=f"vl{lv}_{cg}")
                    ring = nc.sync if cg % 2 == 0 else nc.scalar
                    ring.dma_start(t[:], vt_re[cg][:, col : col + width])
                    vlv[(lv, cg)] = (t, col)
                col += width

            out_re = out.ap().rearrange("(g n p) d -> g p n d", p=128, n=OSUB)

            lvl_of = []              # tile index -> (level, col0)
            c0 = 0
            for lv, width in enumerate(LEVELS):
                for _ in range(width // 128):
                    lvl_of.append((lv, c0))
                c0 += width

            def u_src(i, cg):
                lv, base = lvl_of[i]
                off = i * 128 - base
                t = vth[cg] if lv == 0 else vlv[(lv, cg)][0]
                return t[:, off : off + 128]

            def u_mm_w2(cg):
                return w2a_sb[:] if cg == 0 else w2b_sb[:, cg - 1, :]

            def u_tile(i):
                ups = ups_pool.tile([128, D], F32, tag="ups", name=f"ups{i}")
                for cg in range(4):
                    nc.tensor.matmul(
                        ups[:],
                        u_src(i, cg),
                        u_mm_w2(cg),
                        start=(cg == 0),
                        stop=(cg == 3),
                    )
                # PSUM -> SBUF with the bias folded in and a bf16 downcast
                usb = u_pool.tile([128, D], BF16, tag="usb", name=f"usb{i}")
                nc.vector.tensor_add(usb[:], ups[:], bias_sb[:])
                return usb

            U = {}
            for i in range(PRE_U):
                U[i] = u_tile(i)
            u_first = singles.tile([128, D], BF16)
            nc.vector.tensor_copy(u_first[:], U[0][:])

            ot_tiles = {}
            for i in range(NBLK):
                g, n4 = divmod(i, OSUB)
                if g not in ot_tiles:
                    ot_tiles[g] = opool.tile(
                        [128, OSUB, D], BF16, tag="out", name=f"ot{g}"
                    )
                if i + PRE_U < NBLK:
                    U[i + PRE_U] = u_tile(i + PRE_U)
                u_n = U[i + 1] if i < NBLK - 1 else u_first
                ops = ops_pool.tile([128, D], F32, tag="ops", name=f"ops{i}")
                nc.tensor.matmul(
                    ops[:], band_sb[:, 0, :], U[i][:], start=True, stop=False
                )
                nc.tensor.matmul(
                    ops[:], band_sb[:, 1, :], u_n[:], start=False, stop=True
                )
                del U[i]
                ot = ot_tiles[g]
                nc.scalar.copy(ot[:, n4, :], ops[:])  # ACT: fp32 PSUM -> bf16 SBUF
                if n4 == OSUB - 1:
                    nc.sync.dma_start(out_re[g], ot[:])
                    del ot_tiles[g]

    nc.compile()
    return nc


_RUN_COUNTER = [0]


def _run(nc, in_maps, phase):
    kwargs = {}
    if PROFILE:
        kwargs["trace"] = True
        if TRACE_DIR is not None:
            import os

            _RUN_COUNTER[0] += 1
            d = os.path.join(TRACE_DIR, f"{phase}_{_RUN_COUNTER[0]}")
            os.makedirs(d, exist_ok=True)
            kwargs["tmpdir"] = d
    res = run_bass_kernel_spmd(nc, in_maps, core_ids=list(range(NCORES)), **kwargs)
    LAST_HW_TIME_NS[phase] = res.exec_time_ns
    return res.results


def kernel(q, k, v, Wq, bq, Wk, bk, Wv, bv, Wo, bo):
    q = np.asarray(q, dtype=np.float32)
    k = np.asarray(k, dtype=np.float32)
    v = np.asarray(v, dtype=np.float32)
    Wq, bq, Wk, bk, Wv, bv, Wo, bo = (
        np.asarray(x, dtype=np.float64) for x in (Wq, bq, Wk, bk, Wv, bv, Wo, bo)
    )

    # ---- phase 1: per-batch column sums of q and k (device) ----
    if "p1" not in _NC_CACHE:
        _NC_CACHE["p1"] = _build_phase1()
    q_bf = q.astype(NP_BF16)
    k_bf = k.astype(NP_BF16)
    in_maps = [{"q": q_bf[b], "k": k_bf[b]} for b in range(B)]
    res1 = _run(_NC_CACHE["p1"], in_maps, "phase1")
    sq = np.stack([res1[b]["sums"][0, :D] for b in range(B)]).astype(np.float64)
    sk = np.stack([res1[b]["sums"][0, D:] for b in range(B)]).astype(np.float64)

    # ---- host glue: top-k channel selection + softmax weights ----
    SQ = sq @ Wq + L * bq                       # [B, D]
    SK = sk @ Wk + L * bk
    m = (SQ.reshape(B, H, DK) * SK.reshape(B, H, DK)).sum(axis=1) / (H * L)  # [B, DK]
    mbar = m.mean(axis=0)
    idx = np.argsort(-mbar, kind="stable")[:K_TOP]
    msel = m[:, idx]
    e = np.exp(msel - msel.max(axis=1, keepdims=True))
    w = e / e.sum(axis=1, keepdims=True)        # [B, K_TOP]
    coef = np.zeros((B, DK))
    coef[:, idx] = w

    # Toeplitz bands: out[t] = sum_d coef[d] * U[(t + d) % L]
    s = np.arange(128)[:, None]
    t = np.arange(128)[None, :]
    d1 = s - t
    d2 = s + 128 - t
    m1 = (d1 >= 0) & (d1 < DK)
    m2 = (d2 >= 0) & (d2 < DK)
    bands = np.zeros((B, 2, 128, 128), dtype=np.float64)
    for b in range(B):
        bands[b, 0] = np.where(m1, coef[b][np.clip(d1, 0, DK - 1)], 0.0)
        bands[b, 1] = np.where(m2, coef[b][np.clip(d2, 0, DK - 1)], 0.0)

    W2 = (Wv @ Wo).astype(np.float32)
    bias2 = (bv @ Wo + bo).astype(np.float32)
    bias_rep = np.ascontiguousarray(np.broadcast_to(bias2, (128, D)))
    # split + swizzle: w2a = rows 0..128; w2b rows 128..512 contiguous per chunk
    w2a_bf = np.ascontiguousarray(W2[0:128]).astype(NP_BF16)
    w2b_bf = np.ascontiguousarray(
        W2[128:].reshape(3, 128, D).transpose(1, 0, 2).reshape(128, 3 * D)
    ).astype(NP_BF16)
    bands_bf = bands.astype(NP_BF16)
    vT_bf = np.ascontiguousarray(v.transpose(0, 2, 1)).astype(NP_BF16)  # [B, D, L]

    # ---- phase 2: folded projection + tap aggregation (device) ----
    if "p2" not in _NC_CACHE:
        _NC_CACHE["p2"] = _build_phase2()
    in_maps = [
        {
            "vT": vT_bf[b],
            "bands": np.ascontiguousarray(bands_bf[b]),
            "w2a": w2a_bf,
            "w2b": w2b_bf,
            "bias": bias_rep,
        }
        for b in range(B)
    ]
    res2 = _run(_NC_CACHE["p2"], in_maps, "phase2")
    return np.stack([res2[b]["out"] for b in range(B)]).astype(np.float32)
